# revision 15
# baseline (speedup 1.0000x reference)
"""DualAttention (DANet position+channel attention) on 8 TRN2 NeuronCores.

Sharding: core c handles sample b=c//2, query-half h=c%2 (2048 of 4096
spatial positions). Each core gets its sample's full xf=[512,4096] with its
own half's columns FIRST (attention sums over keys/positions are
permutation-invariant, so column order only matters for which queries the
core computes). BN batch stats are combined with a 4KB AllReduce over all
8 cores (local/per-sample stats fail the 2e-2 gate: 4.9e-2 / 3.1e-2).

Per-core pipeline (v3):
  A: x streamed in 512-col blocks through a 2-buf rotating f32 tile.
     Per block: x8 (fp8 copy, ACT chunks 0-1 / GpSimd chunks 2-3) and
     xbf (bf16 copy, DVE) are made; q,k = wq8/wk8 @ x8 (fp8 DoubleRow);
     D1 ce partial sums from bf16 PE-transposes of xbf (single-pass,
     vs 2x-cost fp32 LOW_HIGH) -> DVE bf16 evac -> bf16 matmul,
     transpose chunk nt+1 pipelined ahead of chunk nt's matmuls; and
     vT = wv8.T @ x8 (fp8 DoubleRow), lagging one block, evacs
     alternating ACT/DVE. All weights arrive pre-cast from the host
     (wq8/wk8/wv8 fp8, wf f32r, ident bf16) - no on-device weight casts.
  D2: channel softmax rows exp(rowmin - ce) (stable form of
      softmax(rowmax - ce)); output bf16 directly from ACT with
      accum_out rowsums; gamma_ca and 1/rowsum folded in on DVE.
      First position-attention E^T pairs run on PE under D2's ACT work.
  D3: cattn^T via bf16 PE transpose.
  E (per 512-query group, software-pipelined; PE-paced):
      E^T[m,q] = k_chunk^T q (bf16, two chunks packed in PE row halves
      via tile_position, concurrent);
      st = exp(E - 7) -> fp8e4 on ACT (the shift cancels in the
      pa/rowsum ratio and keeps st inside e4m3 range for every query);
      PV + rowsum in fp8 DoubleRow (2 m-chunks per instruction);
      ca(g+1) = cattn^T.T @ xbf matmul chains interleaved one-per-pair
      into the PE idle slots; evacuated as ca+2x (DVE + Pool add of the
      bf16 x copy);
      epilogue s = (gpa/rowsum)*pv + (ca+2x) - gpa folded into the
      reciprocal, 2 DVE ops per chunk;
      fuse y(g) = wf_r.T @ s(g) (f32r) emitted during group g+1's
      epilogue, evacuated at the top of group g+2: ACT Copy+accum_out
      (y_sb + BN sum) and ACT Square+accum_out (BN sumsq) for 3 chunks,
      4th chunk on DVE, so neither engine stalls the exp stream.
  F: stats [128,8] via 4KB DRAM AllReduce (floor ~10us; the old
     AllGather + strided [P,8,8] gather-back DMA cost ~48us);
     a=scale*rsqrt(var+eps) (single ACT Rsqrt), b=bias-mean*a;
     relu(y*a+b) alternating ACT (1-op Relu scale/bias) and DVE
     (2-op tensor_scalar), stores split over the SP and ACT DMA queues.
"""
import numpy as np

B, C, H, W = 4, 512, 64, 64
N = H * W            # 4096
C8 = C // 8          # 64
NCORES = 8
NH = N // 2          # 2048 queries per core
P = 128
NB = 512             # free-dim block
MT = N // P          # 32 m-chunks
NG = NH // NB        # 4 query groups
PAIRS = MT // 2      # 16 m-chunk pairs per group
BN_EPS = 1e-5
CNT = float(B * H * W)  # BN count per channel = 16384
C_SHIFT = 7.0        # exp(E - C_SHIFT): keeps st in fp8e4 range

_CACHE = {}


def _build_program():
    import concourse.tile as tile
    from concourse import bacc, mybir
    f32 = mybir.dt.float32
    f32r = mybir.dt.float32r
    bf16 = mybir.dt.bfloat16
    fp8 = mybir.dt.float8e4
    AX = mybir.AxisListType.X
    OP = mybir.AluOpType
    AF = mybir.ActivationFunctionType
    DR = mybir.MatmulPerfMode.DoubleRow

    nc = bacc.Bacc("TRN2", target_bir_lowering=False, debug=False,
                   num_devices=NCORES)

    xl_ap = nc.dram_tensor("xl", [C, N], f32, kind="ExternalInput").ap()
    wqt_ap = nc.dram_tensor("wqt", [C, C8], fp8, kind="ExternalInput").ap()
    wkt_ap = nc.dram_tensor("wkt", [C, C8], fp8, kind="ExternalInput").ap()
    wvt_ap = nc.dram_tensor("wvt", [C, C], fp8, kind="ExternalInput").ap()
    wft_ap = nc.dram_tensor("wft", [C, C], f32r, kind="ExternalInput").ap()
    id_ap = nc.dram_tensor("identb", [P, P], bf16, kind="ExternalInput").ap()
    gca_ap = nc.dram_tensor("gca", [P, 1], f32, kind="ExternalInput").ap()
    bnw_ap = nc.dram_tensor("bnw", [P, 4], f32, kind="ExternalInput").ap()
    bnb_ap = nc.dram_tensor("bnb", [P, 4], f32, kind="ExternalInput").ap()
    y_ap = nc.dram_tensor("y", [C, NH], f32, kind="ExternalOutput").ap()

    xl_r = xl_ap.rearrange("(i p) n -> i p n", p=P)      # [4,128,4096]
    wqt_r = wqt_ap.rearrange("(i p) o -> p i o", p=P)    # [128,4,64]
    wkt_r = wkt_ap.rearrange("(i p) o -> p i o", p=P)
    wvt_r = wvt_ap.rearrange("(i p) o -> p i o", p=P)    # [128,4,512]
    wft_r = wft_ap.rearrange("(i p) o -> p i o", p=P)

    with tile.TileContext(nc) as tc:
        from contextlib import ExitStack
        with ExitStack() as _stk:
            _p = lambda *a, **k: _stk.enter_context(tc.tile_pool(*a, **k))
            consts = _p(name="consts", bufs=1)
            xr_pool = _p(name="xr", bufs=2)
            big = _p(name="big", bufs=1)
            x8r = _p(name="x8r", bufs=3)
            w8k = _p(name="w8k", bufs=3)
            xbf_pool = _p(name="xbf", bufs=2)
            catT_pool = _p(name="catT", bufs=1)
            cef_pool = _p(name="cef", bufs=1)
            sfp = _p(name="sfp", bufs=2)
            stp = _p(name="stp", bufs=8)
            small = _p(name="small", bufs=3)
            small2 = _p(name="small2", bufs=2)
            outst = _p(name="outst", bufs=10)
            misc = _p(name="misc", bufs=1)
            pmm = _p(name="pmm", bufs=3, space="PSUM")
            pacc = _p(name="pacc", bufs=4, space="PSUM")
            prow = _p(name="prow", bufs=1, space="PSUM")
            dram = _p(name="dram", bufs=1, space="DRAM")

            # ---------------- consts (DMA-ordered around the x stream) -----
            wq8 = consts.tile([P, 4, C8], fp8)
            wk8 = consts.tile([P, 4, C8], fp8)
            ident_bf = consts.tile([P, P], bf16)
            gca = consts.tile([P, 1], f32)
            bnw = consts.tile([P, 4], f32)
            bnb = consts.tile([P, 4], f32)
            wf_r = consts.tile([P, 4, NB], f32r)
            wv8 = consts.tile([P, 4, NB], fp8)
            # host folds 4*gamma_pa into wv (exact-in-fp8 x4 keeps the tiny
            # weights out of e4m3 subnormals); ones8=4.0 matches so
            # pv/rowsum = gamma_pa * pa exactly
            ones8 = consts.tile([P, 2, P], fp8)
            nc.vector.memset(ones8[:], 4.0)
            shift_t = consts.tile([P, 1], f32)
            nc.vector.memset(shift_t[:], -C_SHIFT)

            xbf_all = big.tile([P, 4, N], bf16)
            k_bf = big.tile([P, N], bf16)
            q_bf = big.tile([P, NH], bf16)
            vT8 = big.tile([P, MT, NB], fp8)
            y_sb = big.tile([P, 4, NH], bf16)

            ce_acc = [pacc.tile([P, NB], f32, tag="acc", name=f"ce{ct}")
                      for ct in range(4)]
            ce_full = cef_pool.tile([P, 4, NB], f32)
            cein = dram.tile([4 * P, NB], f32)
            ceout = dram.tile([4 * P, NB], f32)
            cein_ap = cein[:]

            # -------- A: stream x; per 512-col block do k,q,ce; vT lags
            # one block so its weight load hides under the x stream ----------
            def emit_vt(x8, nb):
                for mt in range(4 * nb, 4 * nb + 4):
                    lo = (mt - 4 * nb) * P
                    ps = pmm.tile([P, NB], f32, tag="mm", name=f"vt{mt}")
                    for ip in range(2):
                        nc.tensor.matmul(
                            ps[:], x8[:, 2 * ip:2 * ip + 2, lo:lo + P],
                            wv8[:, 2 * ip:2 * ip + 2, :],
                            start=(ip == 0), stop=(ip == 1), perf_mode=DR)
                    if mt % 2 == 0:
                        nc.scalar.activation(out=vT8[:, mt, :], in_=ps[:],
                                             func=AF.Copy, bias=0.0, scale=1.0)
                    else:
                        nc.vector.tensor_copy(vT8[:, mt, :], ps[:])

            x8_prev = None
            xt_pend = None
            for nb in range(8):
                nbs = slice(nb * NB, (nb + 1) * NB)
                X = xr_pool.tile([P, 4, NB], f32, tag="x", name=f"x{nb}")
                if nb == 0:
                    # tiny weight loads first so the first qk is never
                    # gated on them
                    nc.sync.dma_start(wq8[:], wqt_r)
                    nc.sync.dma_start(wk8[:], wkt_r)
                    nc.sync.dma_start(ident_bf[:], id_ap)
                nc.sync.dma_start(X[:, 0:2, :], xl_r[0:2, :, nbs].rearrange(
                    "i p n -> p i n"))
                nc.sync.dma_start(X[:, 2:4, :], xl_r[2:4, :, nbs].rearrange(
                    "i p n -> p i n"))
                if nb == 1:
                    nc.sync.dma_start(wv8[:], wvt_r)
                elif nb == 2:
                    nc.sync.dma_start(wf_r[:], wft_r)
                elif nb == 3:
                    nc.sync.dma_start(gca[:], gca_ap)
                    nc.sync.dma_start(bnw[:], bnw_ap)
                    nc.sync.dma_start(bnb[:], bnb_ap)
                # fp8 conversion in channel-pair halves: the first qk
                # DoubleRow matmul only needs chunks 0-1, so ACT does those
                # first; GpSimd (slow but otherwise idle) takes 2-3
                x8 = x8r.tile([P, 4, NB], fp8, tag="x8", name=f"x8_{nb}")
                nc.scalar.activation(out=x8[:, 0, :], in_=X[:, 0, :],
                                     func=AF.Copy, bias=0.0, scale=1.0)
                nc.scalar.activation(out=x8[:, 1, :], in_=X[:, 1, :],
                                     func=AF.Copy, bias=0.0, scale=1.0)
                nc.scalar.activation(out=x8[:, 2, :], in_=X[:, 2, :],
                                     func=AF.Copy, bias=0.0, scale=1.0)
                nc.gpsimd.tensor_copy(x8[:, 3, :], X[:, 3, :])
                # bf16 copy (ce transposes + ca matmuls + the s=+2x adds)
                nc.vector.tensor_copy(xbf_all[:, 0:2, nbs], X[:, 0:2, :])
                nc.vector.tensor_copy(xbf_all[:, 2:4, nbs], X[:, 2:4, :])

                def emit_qk(dst, w8):
                    ps = pmm.tile([P, NB], f32, tag="mm", name=f"qk{nb}")
                    for ip in range(2):
                        nc.tensor.matmul(ps[:C8, :],
                                         w8[:, 2 * ip:2 * ip + 2, :],
                                         x8[:, 2 * ip:2 * ip + 2, :],
                                         start=(ip == 0), stop=(ip == 1),
                                         perf_mode=DR)
                    nc.scalar.activation(out=dst[:C8, nbs], in_=ps[:C8, :],
                                         func=AF.Copy, bias=0.0, scale=1.0)

                emit_qk(k_bf, wk8)
                if nb < 4:
                    emit_qk(q_bf, wq8)

                # D1 partial, software-pipelined: transpose chunk nt+1 is
                # emitted before chunk nt's ce matmuls so PE never waits on
                # the DVE evacuation of the transpose PSUM
                def emit_xt(nt):
                    tp = pmm.tile([P, NB], bf16, tag="mm", name=f"cetp{nt}")
                    for i in range(4):
                        nc.tensor.transpose(tp[:, i * P:(i + 1) * P],
                                            xbf_all[:, i,
                                                    nt * P:(nt + 1) * P],
                                            ident_bf[:])
                    xt = small.tile([P, NB], bf16, tag="xt", name=f"xt{nt}")
                    nc.vector.tensor_copy(xt[:], tp[:])
                    return xt

                def emit_ce_mm(pnt, pxt, stop):
                    for ct in range(4):
                        nc.tensor.matmul(ce_acc[ct],
                                         pxt[:, ct * P:(ct + 1) * P],
                                         pxt[:],
                                         start=(pnt == 0), stop=stop)

                def emit_ce_exchange():
                    # ce partial covered OWN half columns only; the pair
                    # core covers the other half (its blocks 0-3); combine
                    # with a pairwise 1MB AllReduce hidden under blocks 4-7
                    ce_sb = w8k.tile([P, 4, NB], f32, tag="w8",
                                     name="ce_sb")
                    for ct in range(4):
                        if ct % 2 == 0:
                            nc.scalar.activation(out=ce_sb[:, ct, :],
                                                 in_=ce_acc[ct],
                                                 func=AF.Copy, bias=0.0,
                                                 scale=1.0)
                        else:
                            nc.vector.tensor_copy(ce_sb[:, ct, :],
                                                  ce_acc[ct])
                    nc.gpsimd.dma_start(
                        cein_ap.rearrange("(i p) c -> p i c", p=P),
                        ce_sb[:])
                    nc.gpsimd.collective_compute(
                        "AllReduce", OP.add,
                        replica_groups=[[2 * i, 2 * i + 1]
                                        for i in range(NCORES // 2)],
                        ins=[cein.opt()], outs=[ceout.opt()],
                    )

                if nb < 4:
                    for nt in range(4 * nb, 4 * nb + 4):
                        xt = emit_xt(nt)
                        if xt_pend is not None:
                            emit_ce_mm(xt_pend[0], xt_pend[1], False)
                        xt_pend = (nt, xt)
                if nb == 3:
                    emit_ce_mm(xt_pend[0], xt_pend[1], True)
                    xt_pend = None
                    emit_ce_exchange()

                if x8_prev is not None:
                    emit_vt(x8_prev, nb - 1)
                x8_prev = x8

                # pack for row-tiled E^T: k rows 64-127 = k shifted left one
                # 128-chunk (so pair (2p, 2p+1) computes in one PE pass);
                # q rows 64-127 = copy of q. Emitted as soon as the source
                # columns exist so group-0 pairs can start early.
                if nb == 3:
                    nc.sync.dma_start(q_bf[C8:P, :], q_bf[0:C8, :])
                elif nb == 4:
                    nc.sync.dma_start(k_bf[C8:P, 0:NH],
                                      k_bf[0:C8, P:NH + P])
                elif nb == 7:
                    nc.sync.dma_start(k_bf[C8:P, NH:N - P],
                                      k_bf[0:C8, NH + P:N])
            emit_vt(x8_prev, 7)
            # pull in the pair-combined ce (sync queue is idle here; the
            # wait on the collective overlaps the stocked g0 pairs)
            nc.sync.dma_start(ce_full[:],
                              ceout[:].rearrange("(i p) c -> p i c", p=P))

            # ---------------- E^T pair machinery ----------------
            def emit_pair(g, p):
                gs = slice(g * NB, (g + 1) * NB)
                mt = 2 * p
                p1 = pmm.tile([P, NB], f32, tag="mm", name=f"e{g}_{mt}")
                p2 = pmm.tile([P, NB], f32, tag="mm", name=f"e{g}_{mt + 1}")
                nc.tensor.matmul(p1[:], k_bf[0:C8, mt * P:(mt + 1) * P],
                                 q_bf[0:C8, gs], start=True, stop=True,
                                 tile_position=(0, 0))
                nc.tensor.matmul(p2[:], k_bf[C8:P, mt * P:(mt + 1) * P],
                                 q_bf[C8:P, gs], start=True, stop=True,
                                 tile_position=(64, 0))
                st = stp.tile([P, 2, NB], fp8, tag="st", name=f"st{g}_{p}")
                nc.scalar.activation(out=st[:, 0, :], in_=p1[:], func=AF.Exp,
                                     bias=shift_t[:, 0:1], scale=1.0)
                nc.scalar.activation(out=st[:, 1, :], in_=p2[:], func=AF.Exp,
                                     bias=shift_t[:, 0:1], scale=1.0)
                return st

            # 6 pairs of group 0 run on PE while ACT does D2 below
            g0_sts = [emit_pair(0, p) for p in range(6)]

            # ---------------- D2: channel softmax (bf16 out) ----------------
            cattn_sb = xbf_pool.tile([P, 4, NB], bf16, tag="xbf",
                                     name="cattn_sb")
            cmin = misc.tile([P, 4], f32)
            csum = misc.tile([P, 4], f32)
            for ct in range(4):
                nc.vector.tensor_reduce(out=cmin[:, ct:ct + 1],
                                        in_=ce_full[:, ct, :], axis=AX,
                                        op=OP.min)
                nc.scalar.activation(out=cattn_sb[:, ct, :],
                                     in_=ce_full[:, ct, :],
                                     func=AF.Exp, bias=cmin[:, ct:ct + 1],
                                     scale=-1.0,
                                     accum_out=csum[:, ct:ct + 1])
            crcp = misc.tile([P, 4], f32)
            nc.vector.reciprocal(crcp[:], csum[:])
            nc.vector.tensor_scalar(out=crcp[:], in0=crcp[:],
                                    scalar1=gca[:, 0:1], scalar2=None,
                                    op0=OP.mult)
            cattn_bf = xbf_pool.tile([P, 4, NB], bf16, tag="xbf",
                                     name="cattn_bf")
            for ct in range(4):
                nc.vector.tensor_scalar(out=cattn_bf[:, ct, :],
                                        in0=cattn_sb[:, ct, :],
                                        scalar1=crcp[:, ct:ct + 1],
                                        scalar2=None, op0=OP.mult)

            # ---------------- D3: cattn^T (bf16) ----------------
            catT = catT_pool.tile([P, 4, NB], bf16)
            for dt in range(4):
                tp = pmm.tile([P, NB], bf16, tag="mm", name=f"catp{dt}")
                for ct in range(4):
                    nc.tensor.transpose(tp[:, ct * P:(ct + 1) * P],
                                        cattn_bf[:, ct, dt * P:(dt + 1) * P],
                                        ident_bf[:])
                nc.vector.tensor_copy(catT[:, dt, :], tp[:])

            # ---------------- E: position attention ----------------
            ysum = misc.tile([P, 4, 4], f32)
            ysq = misc.tile([P, 4, 4], f32)

            def emit_ca(g, ca2x):
                for p in range(16):
                    emit_ca_step(g, ca2x, p)
                return ca2x

            ca_state = {}

            def emit_ca_step(g, ca2x, p):
                gsc = slice(g * NB, (g + 1) * NB)
                ct, dt = divmod(p, 4)
                if dt == 0:
                    ca_state["cp"] = pmm.tile([P, NB], f32, tag="mm",
                                              name=f"cap{g}_{ct}")
                cp = ca_state["cp"]
                nc.tensor.matmul(cp[:], catT[:, dt, ct * P:(ct + 1) * P],
                                 xbf_all[:, dt, gsc],
                                 start=(dt == 0), stop=(dt == 3))
                if dt == 3:
                    nc.vector.tensor_tensor(out=ca2x[:, ct, :], in0=cp[:],
                                            in1=xbf_all[:, ct, gsc],
                                            op=OP.add)
                    nc.gpsimd.tensor_tensor(out=ca2x[:, ct, :],
                                            in0=ca2x[:, ct, :],
                                            in1=xbf_all[:, ct, gsc],
                                            op=OP.add)

            def emit_sq(g, ot):
                # deferred BN sum-of-squares, reading the SBUF y copy so it
                # never holds PSUM banks; ACT for even ot, DVE for odd
                gsf = slice(g * NB, (g + 1) * NB)
                if ot % 2 == 0:
                    sqb = small2.tile([P, NB], f32, tag="sq",
                                      name=f"sq{g}_{ot}")
                    nc.scalar.activation(out=sqb[:], in_=y_sb[:, ot, gsf],
                                         func=AF.Square, bias=0.0, scale=1.0,
                                         accum_out=ysq[:, ot, g:g + 1])
                else:
                    sqb = small2.tile([P, NB], f32, tag="sq",
                                      name=f"sq{g}_{ot}")
                    nc.vector.tensor_tensor(out=sqb[:], in0=y_sb[:, ot, gsf],
                                            in1=y_sb[:, ot, gsf], op=OP.mult)
                    nc.vector.tensor_reduce(out=ysq[:, ot, g:g + 1],
                                            in_=sqb[:], axis=AX, op=OP.add)

            ca_cur = w8k.tile([P, 4, NB], f32, tag="w8", name="ca0")
            emit_ca(0, ca_cur)
            sts = g0_sts
            defer_sq = []
            for g in range(NG):
                pv = [pacc.tile([P, NB], f32, tag="acc", name=f"pv{g}_{ct}")
                      for ct in range(4)]
                pr_ = prow.tile([P, NB], f32, tag="rs", name=f"rs{g}")
                ca_next = None
                if g + 1 < NG:
                    ca_next = w8k.tile([P, 4, NB], f32, tag="w8",
                                       name=f"ca{g + 1}")
                for p in range(PAIRS):
                    while len(sts) < min(p + 5, PAIRS):
                        sts.append(emit_pair(g, len(sts)))
                    st = sts[p]
                    for ct in range(4):
                        nc.tensor.matmul(
                            pv[ct],
                            vT8[:, 2 * p:2 * p + 2, ct * P:(ct + 1) * P],
                            st[:], start=(p == 0), stop=(p == PAIRS - 1),
                            perf_mode=DR)
                    nc.tensor.matmul(pr_, ones8[:], st[:],
                                     start=(p == 0), stop=(p == PAIRS - 1),
                                     perf_mode=DR)
                    # next group's ca chain + the previous group's deferred
                    # squares ride the PE/ACT/DVE slack of the pair loop
                    if ca_next is not None:
                        emit_ca_step(g + 1, ca_next, p)
                    if defer_sq and p in (4, 6, 8, 10):
                        defer_sq.pop(0)()

                # epilogue: ca(g+1) tail + next group's first pairs keep PE
                # busy under the DVE reciprocal; the s0 reads then release
                # the PV banks one-by-one, and this group's fuse matmuls
                # recycle them per-ot with an immediate alternating DVE/ACT
                # evac, so no separate fuse/evac stage blocks group g+1
                next_sts = []
                if g + 1 < NG:
                    next_sts = [emit_pair(g + 1, p) for p in range(4)]

                rr = small2.tile([P, NB], f32, tag="sq", name=f"rr{g}")
                nc.vector.reciprocal(rr[:], pr_)
                ca_g = ca_cur
                ca_cur = ca_next
                s_f = sfp.tile([P, 4, NB], f32r, tag="sf", name=f"sf{g}")
                s0 = w8k.tile([P, 4, NB], f32, tag="w8", name=f"s0_{g}")
                for ct in range(4):
                    nc.vector.tensor_tensor(out=s0[:, ct, :],
                                            in0=pv[ct], in1=rr[:],
                                            op=OP.mult)
                    nc.vector.tensor_tensor(out=s_f[:, ct, :],
                                            in0=s0[:, ct, :],
                                            in1=ca_g[:, ct, :], op=OP.add)

                gsf = slice(g * NB, (g + 1) * NB)
                for ot in range(4):
                    yp = pacc.tile([P, NB], f32, tag="acc",
                                   name=f"y{g}_{ot}")
                    for ct in range(4):
                        nc.tensor.matmul(yp,
                                         wf_r[:, ct, ot * P:(ot + 1) * P],
                                         s_f[:, ct, :],
                                         start=(ct == 0), stop=(ct == 3))
                    if ot % 2 == 0:
                        nc.vector.tensor_copy(y_sb[:, ot, gsf], yp)
                        nc.vector.tensor_reduce(out=ysum[:, ot, g:g + 1],
                                                in_=y_sb[:, ot, gsf],
                                                axis=AX, op=OP.add)
                    else:
                        nc.scalar.activation(out=y_sb[:, ot, gsf],
                                             in_=yp, func=AF.Copy,
                                             bias=0.0, scale=1.0,
                                             accum_out=ysum[:, ot, g:g + 1])
                    defer_sq.append(
                        (lambda gg, oo: lambda: emit_sq(gg, oo))(g, ot))
                sts = next_sts

            # last group's squares never got a pair loop to ride
            while defer_sq:
                defer_sq.pop(0)()

            # ---------------- F: BN via AllReduce + apply ----------------
            stats = misc.tile([P, 8, 1], f32)
            nc.vector.tensor_reduce(out=stats[:, 0:4, :],
                                    in_=ysum[:], axis=AX, op=OP.add)
            nc.vector.tensor_reduce(out=stats[:, 4:8, :],
                                    in_=ysq[:], axis=AX, op=OP.add)
            stats_f = stats.rearrange("p a b -> p (a b)")
            sin = dram.tile([NCORES, P * 8], f32)
            sout = dram.tile([NCORES, P * 8], f32)
            sin_r = sin.rearrange("r (p j) -> r p j", p=P)
            for r in range(NCORES):
                q = nc.sync if r % 2 == 0 else nc.scalar
                q.dma_start(sin_r[r], stats_f)
            # 8-core AllToAll of the replicated stats == AllGather but one
            # hop on the CC cores (~5us floor) instead of a 7-step ring
            nc.gpsimd.collective_compute(
                "AllToAll", OP.bypass,
                replica_groups=[list(range(NCORES))],
                ins=[sin.opt()], outs=[sout.opt()],
            )
            # per-rank contiguous [P,8] loads (the one-shot [P,8,R] gather
            # is 4B-strided and costs ~12us in DMA packets), then a short
            # DVE rank-sum tree
            gst = misc.tile([P, NCORES, 8], f32)
            sout_r = sout.rearrange("r (p j) -> r p j", p=P)
            for r in range(NCORES):
                q = nc.sync if r % 2 == 0 else nc.scalar
                q.dma_start(gst[:, r, :], sout_r[r])
            for r in range(1, NCORES):
                nc.vector.tensor_tensor(out=gst[:, 0, :], in0=gst[:, 0, :],
                                        in1=gst[:, r, :], op=OP.add)
            mean = misc.tile([P, 4], f32)
            msq = misc.tile([P, 4], f32)
            nc.vector.tensor_scalar(out=mean[:], in0=gst[:, 0, 0:4],
                                    scalar1=1.0 / CNT, scalar2=None,
                                    op0=OP.mult)
            nc.vector.tensor_scalar(out=msq[:], in0=gst[:, 0, 4:8],
                                    scalar1=1.0 / CNT, scalar2=None,
                                    op0=OP.mult)
            var = misc.tile([P, 4], f32)
            nc.vector.tensor_tensor(out=var[:], in0=mean[:], in1=mean[:],
                                    op=OP.mult)
            nc.vector.tensor_tensor(out=var[:], in0=msq[:], in1=var[:],
                                    op=OP.subtract)
            eps_t = misc.tile([P, 1], f32)
            nc.vector.memset(eps_t[:], BN_EPS)
            sd = misc.tile([P, 4], f32)
            nc.scalar.activation(out=sd[:], in_=var[:], func=AF.Sqrt,
                                 bias=eps_t[:, 0:1], scale=1.0)
            rstd = misc.tile([P, 4], f32)
            nc.vector.reciprocal(rstd[:], sd[:])
            a_t = misc.tile([P, 4], f32)
            nc.vector.tensor_tensor(out=a_t[:], in0=bnw[:], in1=rstd[:],
                                    op=OP.mult)
            b_t = misc.tile([P, 4], f32)
            nc.vector.tensor_tensor(out=b_t[:], in0=mean[:], in1=a_t[:],
                                    op=OP.mult)
            nc.vector.tensor_tensor(out=b_t[:], in0=bnb[:], in1=b_t[:],
                                    op=OP.subtract)

            # apply relu(y*a+b) alternating ACT/DVE; stores balanced 10:6
            # over the SP and ACT DMA queues
            for idx in range(16):
                ot, gg = divmod(idx, 4)
                ggs = slice(gg * NB, (gg + 1) * NB)
                ost = outst.tile([P, NB], f32, tag="ost",
                                 name=f"ost{ot}_{gg}")
                if idx % 2 == 0:
                    nc.scalar.activation(out=ost[:], in_=y_sb[:, ot, ggs],
                                         func=AF.Relu,
                                         scale=a_t[:, ot:ot + 1],
                                         bias=b_t[:, ot:ot + 1])
                else:
                    nc.vector.tensor_scalar(out=ost[:],
                                            in0=y_sb[:, ot, ggs],
                                            scalar1=a_t[:, ot:ot + 1],
                                            scalar2=b_t[:, ot:ot + 1],
                                            op0=OP.mult, op1=OP.add)
                    nc.vector.tensor_scalar(out=ost[:], in0=ost[:],
                                            scalar1=0.0, scalar2=None,
                                            op0=OP.max)
                store = nc.sync if idx % 8 < 5 else nc.scalar
                store.dma_start(y_ap[ot * P:(ot + 1) * P, ggs], ost[:])

    nc.compile()
    return nc


def _make_in_maps(inputs):
    import ml_dtypes
    x = np.asarray(inputs["x"], dtype=np.float32)
    xf = x.reshape(B, C, N)
    gpa_s = np.float32(np.asarray(inputs["gamma_pa"]).reshape(-1)[0])
    gca_s = np.float32(np.asarray(inputs["gamma_ca"]).reshape(-1)[0])
    wqt = np.ascontiguousarray(
        np.asarray(inputs["wq"], np.float32).T).astype(ml_dtypes.float8_e4m3)
    wkt = np.ascontiguousarray(
        np.asarray(inputs["wk"], np.float32).T).astype(ml_dtypes.float8_e4m3)
    wvt = np.ascontiguousarray(
        4.0 * gpa_s * np.asarray(inputs["wv"], np.float32).T
    ).astype(ml_dtypes.float8_e4m3)
    wft = np.ascontiguousarray(np.asarray(inputs["w_fuse"], np.float32).T)
    identb = np.eye(P, dtype=ml_dtypes.bfloat16)
    gca = np.full((P, 1), gca_s, np.float32)
    bnw = np.ascontiguousarray(
        np.asarray(inputs["bn_scale"], np.float32).reshape(4, P).T)
    bnb = np.ascontiguousarray(
        np.asarray(inputs["bn_bias"], np.float32).reshape(4, P).T)

    in_maps = []
    for c in range(NCORES):
        b, h = divmod(c, 2)
        own = xf[b][:, h * NH:(h + 1) * NH]
        other = xf[b][:, (1 - h) * NH:(2 - h) * NH]
        xl = np.ascontiguousarray(np.concatenate([own, other], axis=1))
        in_maps.append({
            "xl": xl, "wqt": wqt, "wkt": wkt, "wvt": wvt, "wft": wft,
            "identb": identb, "gca": gca, "bnw": bnw, "bnb": bnb,
        })
    return in_maps


def kernel(x, wq, wk, wv, w_fuse, gamma_pa, gamma_ca, bn_scale, bn_bias):
    from concourse.bass_utils import run_bass_kernel_spmd

    if "nc" not in _CACHE:
        _CACHE["nc"] = _build_program()
    nc = _CACHE["nc"]

    in_maps = _make_in_maps({
        "x": x, "wq": wq, "wk": wk, "wv": wv, "w_fuse": w_fuse,
        "gamma_pa": gamma_pa, "gamma_ca": gamma_ca,
        "bn_scale": bn_scale, "bn_bias": bn_bias,
    })
    res = run_bass_kernel_spmd(nc, in_maps, core_ids=list(range(NCORES)))
    out = np.empty((B, C, N), dtype=np.float32)
    for c in range(NCORES):
        b, h = divmod(c, 2)
        out[b][:, h * NH:(h + 1) * NH] = res.results[c]["y"]
    return out.reshape(B, C, H, W)


# revision 17
# speedup vs baseline: 1.0174x; 1.0174x over previous
"""DualAttention (DANet position+channel attention) on 8 TRN2 NeuronCores.

Sharding: core c handles sample b=c//2, query-half h=c%2 (2048 of 4096
spatial positions). Each core gets its sample's full xf=[512,4096] with its
own half's columns FIRST (attention sums over keys/positions are
permutation-invariant, so column order only matters for which queries the
core computes). BN batch stats are combined with a 4KB AllReduce over all
8 cores (local/per-sample stats fail the 2e-2 gate: 4.9e-2 / 3.1e-2).

Per-core pipeline (v3):
  A: x streamed in 512-col blocks through a 2-buf rotating f32 tile.
     Per block: x8 (fp8 copy, ACT chunks 0-1 / GpSimd chunks 2-3) and
     xbf (bf16 copy, DVE) are made; q,k = wq8/wk8 @ x8 (fp8 DoubleRow);
     D1 ce partial sums from bf16 PE-transposes of xbf (single-pass,
     vs 2x-cost fp32 LOW_HIGH) -> DVE bf16 evac -> bf16 matmul,
     transpose chunk nt+1 pipelined ahead of chunk nt's matmuls; and
     vT = wv8.T @ x8 (fp8 DoubleRow), lagging one block, evacs
     alternating ACT/DVE. All weights arrive pre-cast from the host
     (wq8/wk8/wv8 fp8, wf f32r, ident bf16) - no on-device weight casts.
  D2: channel softmax rows exp(rowmin - ce) (stable form of
      softmax(rowmax - ce)); output bf16 directly from ACT with
      accum_out rowsums; gamma_ca and 1/rowsum folded in on DVE.
      First position-attention E^T pairs run on PE under D2's ACT work.
  D3: cattn^T via bf16 PE transpose.
  E (per 512-query group, software-pipelined; PE-paced):
      E^T[m,q] = k_chunk^T q (bf16, two chunks packed in PE row halves
      via tile_position, concurrent);
      st = exp(E - 7) -> fp8e4 on ACT (the shift cancels in the
      pa/rowsum ratio and keeps st inside e4m3 range for every query);
      PV + rowsum in fp8 DoubleRow (2 m-chunks per instruction);
      ca(g+1) = cattn^T.T @ xbf matmul chains interleaved one-per-pair
      into the PE idle slots; evacuated as ca+2x (DVE + Pool add of the
      bf16 x copy);
      epilogue s = (gpa/rowsum)*pv + (ca+2x) - gpa folded into the
      reciprocal, 2 DVE ops per chunk;
      fuse y(g) = wf_r.T @ s(g) (f32r) emitted during group g+1's
      epilogue, evacuated at the top of group g+2: ACT Copy+accum_out
      (y_sb + BN sum) and ACT Square+accum_out (BN sumsq) for 3 chunks,
      4th chunk on DVE, so neither engine stalls the exp stream.
  F: stats [128,8] via 4KB DRAM AllReduce (floor ~10us; the old
     AllGather + strided [P,8,8] gather-back DMA cost ~48us);
     a=scale*rsqrt(var+eps) (single ACT Rsqrt), b=bias-mean*a;
     relu(y*a+b) alternating ACT (1-op Relu scale/bias) and DVE
     (2-op tensor_scalar), stores split over the SP and ACT DMA queues.
"""
import numpy as np

B, C, H, W = 4, 512, 64, 64
N = H * W            # 4096
C8 = C // 8          # 64
NCORES = 8
NH = N // 2          # 2048 queries per core
P = 128
NB = 512             # free-dim block
MT = N // P          # 32 m-chunks
NG = NH // NB        # 4 query groups
PAIRS = MT // 2      # 16 m-chunk pairs per group
BN_EPS = 1e-5
CNT = float(B * H * W)  # BN count per channel = 16384
C_SHIFT = 7.0        # exp(E - C_SHIFT): keeps st in fp8e4 range

_CACHE = {}


def _build_program():
    import concourse.tile as tile
    from concourse import bacc, mybir
    f32 = mybir.dt.float32
    f32r = mybir.dt.float32r
    bf16 = mybir.dt.bfloat16
    fp8 = mybir.dt.float8e4
    AX = mybir.AxisListType.X
    OP = mybir.AluOpType
    AF = mybir.ActivationFunctionType
    DR = mybir.MatmulPerfMode.DoubleRow

    nc = bacc.Bacc("TRN2", target_bir_lowering=False, debug=False,
                   num_devices=NCORES)

    xl_ap = nc.dram_tensor("xl", [C, N], f32, kind="ExternalInput").ap()
    wqt_ap = nc.dram_tensor("wqt", [C, C8], fp8, kind="ExternalInput").ap()
    wkt_ap = nc.dram_tensor("wkt", [C, C8], fp8, kind="ExternalInput").ap()
    wvt_ap = nc.dram_tensor("wvt", [C, C], fp8, kind="ExternalInput").ap()
    wft_ap = nc.dram_tensor("wft", [C, C], f32r, kind="ExternalInput").ap()
    id_ap = nc.dram_tensor("identb", [P, P], bf16, kind="ExternalInput").ap()
    gca_ap = nc.dram_tensor("gca", [P, 1], f32, kind="ExternalInput").ap()
    bnw_ap = nc.dram_tensor("bnw", [P, 4], f32, kind="ExternalInput").ap()
    bnb_ap = nc.dram_tensor("bnb", [P, 4], f32, kind="ExternalInput").ap()
    y_ap = nc.dram_tensor("y", [C, NH], f32, kind="ExternalOutput").ap()

    xl_r = xl_ap.rearrange("(i p) n -> i p n", p=P)      # [4,128,4096]
    wqt_r = wqt_ap.rearrange("(i p) o -> p i o", p=P)    # [128,4,64]
    wkt_r = wkt_ap.rearrange("(i p) o -> p i o", p=P)
    wvt_r = wvt_ap.rearrange("(i p) o -> p i o", p=P)    # [128,4,512]
    wft_r = wft_ap.rearrange("(i p) o -> p i o", p=P)

    with tile.TileContext(nc) as tc:
        from contextlib import ExitStack
        with ExitStack() as _stk:
            _p = lambda *a, **k: _stk.enter_context(tc.tile_pool(*a, **k))
            consts = _p(name="consts", bufs=1)
            xr_pool = _p(name="xr", bufs=2)
            big = _p(name="big", bufs=1)
            x8r = _p(name="x8r", bufs=3)
            w8k = _p(name="w8k", bufs=3)
            xbf_pool = _p(name="xbf", bufs=2)
            catT_pool = _p(name="catT", bufs=1)
            cef_pool = _p(name="cef", bufs=1)
            sfp = _p(name="sfp", bufs=2)
            stp = _p(name="stp", bufs=8)
            small = _p(name="small", bufs=3)
            small2 = _p(name="small2", bufs=2)
            outst = _p(name="outst", bufs=10)
            misc = _p(name="misc", bufs=1)
            pmm = _p(name="pmm", bufs=3, space="PSUM")
            pacc = _p(name="pacc", bufs=4, space="PSUM")
            prow = _p(name="prow", bufs=1, space="PSUM")
            dram = _p(name="dram", bufs=1, space="DRAM")

            # ---------------- consts (DMA-ordered around the x stream) -----
            wq8 = consts.tile([P, 4, C8], fp8)
            wk8 = consts.tile([P, 4, C8], fp8)
            ident_bf = consts.tile([P, P], bf16)
            gca = consts.tile([P, 1], f32)
            bnw = consts.tile([P, 4], f32)
            bnb = consts.tile([P, 4], f32)
            wf_r = consts.tile([P, 4, NB], f32r)
            wv8 = consts.tile([P, 4, NB], fp8)
            # host folds 4*gamma_pa into wv (exact-in-fp8 x4 keeps the tiny
            # weights out of e4m3 subnormals); ones8=4.0 matches so
            # pv/rowsum = gamma_pa * pa exactly
            ones8 = consts.tile([P, 2, P], fp8)
            nc.vector.memset(ones8[:], 4.0)
            shift_t = consts.tile([P, 1], f32)
            nc.vector.memset(shift_t[:], -C_SHIFT)

            xbf_all = big.tile([P, 4, N], bf16)
            k_bf = big.tile([P, N], bf16)
            q_bf = big.tile([P, NH], bf16)
            vT8 = big.tile([P, MT, NB], fp8)
            y_sb = big.tile([P, 4, NH], bf16)

            ce_acc = [pacc.tile([P, NB], f32, tag="acc", name=f"ce{ct}")
                      for ct in range(4)]

            # -------- A: stream x; per 512-col block do k,q,ce; vT lags
            # one block so its weight load hides under the x stream ----------
            def emit_vt(x8, nb):
                for mt in range(4 * nb, 4 * nb + 4):
                    lo = (mt - 4 * nb) * P
                    ps = pmm.tile([P, NB], f32, tag="mm", name=f"vt{mt}")
                    for ip in range(2):
                        nc.tensor.matmul(
                            ps[:], x8[:, 2 * ip:2 * ip + 2, lo:lo + P],
                            wv8[:, 2 * ip:2 * ip + 2, :],
                            start=(ip == 0), stop=(ip == 1), perf_mode=DR)
                    if mt % 2 == 0:
                        nc.scalar.activation(out=vT8[:, mt, :], in_=ps[:],
                                             func=AF.Copy, bias=0.0, scale=1.0)
                    else:
                        nc.vector.tensor_copy(vT8[:, mt, :], ps[:])

            x8_prev = None
            xt_pend = None
            for nb in range(8):
                nbs = slice(nb * NB, (nb + 1) * NB)
                X = xr_pool.tile([P, 4, NB], f32, tag="x", name=f"x{nb}")
                if nb == 0:
                    # tiny weight loads first so the first qk is never
                    # gated on them
                    nc.sync.dma_start(wq8[:], wqt_r)
                    nc.sync.dma_start(wk8[:], wkt_r)
                    nc.sync.dma_start(ident_bf[:], id_ap)
                nc.sync.dma_start(X[:, 0:2, :], xl_r[0:2, :, nbs].rearrange(
                    "i p n -> p i n"))
                nc.sync.dma_start(X[:, 2:4, :], xl_r[2:4, :, nbs].rearrange(
                    "i p n -> p i n"))
                if nb == 1:
                    nc.sync.dma_start(wv8[:], wvt_r)
                elif nb == 2:
                    nc.sync.dma_start(wf_r[:], wft_r)
                elif nb == 3:
                    nc.sync.dma_start(gca[:], gca_ap)
                    nc.sync.dma_start(bnw[:], bnw_ap)
                    nc.sync.dma_start(bnb[:], bnb_ap)
                # fp8 conversion in channel-pair halves: the first qk
                # DoubleRow matmul only needs chunks 0-1, so ACT does those
                # first; GpSimd (slow but otherwise idle) takes 2-3
                x8 = x8r.tile([P, 4, NB], fp8, tag="x8", name=f"x8_{nb}")
                nc.scalar.activation(out=x8[:, 0, :], in_=X[:, 0, :],
                                     func=AF.Copy, bias=0.0, scale=1.0)
                nc.scalar.activation(out=x8[:, 1, :], in_=X[:, 1, :],
                                     func=AF.Copy, bias=0.0, scale=1.0)
                nc.scalar.activation(out=x8[:, 2, :], in_=X[:, 2, :],
                                     func=AF.Copy, bias=0.0, scale=1.0)
                nc.gpsimd.tensor_copy(x8[:, 3, :], X[:, 3, :])
                # bf16 copy (ce transposes + ca matmuls + the s=+2x adds)
                nc.vector.tensor_copy(xbf_all[:, 0:2, nbs], X[:, 0:2, :])
                nc.vector.tensor_copy(xbf_all[:, 2:4, nbs], X[:, 2:4, :])

                def emit_qk(dst, w8):
                    ps = pmm.tile([P, NB], f32, tag="mm", name=f"qk{nb}")
                    for ip in range(2):
                        nc.tensor.matmul(ps[:C8, :],
                                         w8[:, 2 * ip:2 * ip + 2, :],
                                         x8[:, 2 * ip:2 * ip + 2, :],
                                         start=(ip == 0), stop=(ip == 1),
                                         perf_mode=DR)
                    nc.scalar.activation(out=dst[:C8, nbs], in_=ps[:C8, :],
                                         func=AF.Copy, bias=0.0, scale=1.0)

                emit_qk(k_bf, wk8)
                if nb < 4:
                    emit_qk(q_bf, wq8)

                # D1 partial, software-pipelined: transpose chunk nt+1 is
                # emitted before chunk nt's ce matmuls so PE never waits on
                # the DVE evacuation of the transpose PSUM
                def emit_xt(nt):
                    tp = pmm.tile([P, NB], bf16, tag="mm", name=f"cetp{nt}")
                    for i in range(4):
                        nc.tensor.transpose(tp[:, i * P:(i + 1) * P],
                                            xbf_all[:, i,
                                                    nt * P:(nt + 1) * P],
                                            ident_bf[:])
                    xt = small.tile([P, NB], bf16, tag="xt", name=f"xt{nt}")
                    nc.vector.tensor_copy(xt[:], tp[:])
                    return xt

                def emit_ce_mm(pnt, pxt, stop):
                    for ct in range(4):
                        nc.tensor.matmul(ce_acc[ct],
                                         pxt[:, ct * P:(ct + 1) * P],
                                         pxt[:],
                                         start=(pnt == 0), stop=stop)

                for nt in range(4 * nb, 4 * nb + 4):
                    xt = emit_xt(nt)
                    if xt_pend is not None:
                        emit_ce_mm(xt_pend[0], xt_pend[1], False)
                    xt_pend = (nt, xt)

                if x8_prev is not None:
                    emit_vt(x8_prev, nb - 1)
                x8_prev = x8

                # pack for row-tiled E^T: k rows 64-127 = k shifted left one
                # 128-chunk (so pair (2p, 2p+1) computes in one PE pass);
                # q rows 64-127 = copy of q. Emitted as soon as the source
                # columns exist so group-0 pairs can start early.
                if nb == 3:
                    nc.sync.dma_start(q_bf[C8:P, :], q_bf[0:C8, :])
                elif nb == 4:
                    nc.sync.dma_start(k_bf[C8:P, 0:NH],
                                      k_bf[0:C8, P:NH + P])
                elif nb == 7:
                    nc.sync.dma_start(k_bf[C8:P, NH:N - P],
                                      k_bf[0:C8, NH + P:N])
            emit_ce_mm(xt_pend[0], xt_pend[1], True)
            emit_vt(x8_prev, 7)

            # ---------------- E^T pair machinery ----------------
            def emit_pair(g, p):
                gs = slice(g * NB, (g + 1) * NB)
                mt = 2 * p
                p1 = pmm.tile([P, NB], f32, tag="mm", name=f"e{g}_{mt}")
                p2 = pmm.tile([P, NB], f32, tag="mm", name=f"e{g}_{mt + 1}")
                nc.tensor.matmul(p1[:], k_bf[0:C8, mt * P:(mt + 1) * P],
                                 q_bf[0:C8, gs], start=True, stop=True,
                                 tile_position=(0, 0))
                nc.tensor.matmul(p2[:], k_bf[C8:P, mt * P:(mt + 1) * P],
                                 q_bf[C8:P, gs], start=True, stop=True,
                                 tile_position=(64, 0))
                st = stp.tile([P, 2, NB], fp8, tag="st", name=f"st{g}_{p}")
                nc.scalar.activation(out=st[:, 0, :], in_=p1[:], func=AF.Exp,
                                     bias=shift_t[:, 0:1], scale=1.0)
                nc.scalar.activation(out=st[:, 1, :], in_=p2[:], func=AF.Exp,
                                     bias=shift_t[:, 0:1], scale=1.0)
                return st

            # 6 pairs of group 0 run on PE while ACT does D2 below
            g0_sts = [emit_pair(0, p) for p in range(6)]

            # ---------------- D2: channel softmax (bf16 out) ----------------
            cattn_sb = xbf_pool.tile([P, 4, NB], bf16, tag="xbf",
                                     name="cattn_sb")
            cmin = misc.tile([P, 4], f32)
            csum = misc.tile([P, 4], f32)
            for ct in range(4):
                nc.vector.tensor_reduce(out=cmin[:, ct:ct + 1],
                                        in_=ce_acc[ct], axis=AX, op=OP.min)
                nc.scalar.activation(out=cattn_sb[:, ct, :], in_=ce_acc[ct],
                                     func=AF.Exp, bias=cmin[:, ct:ct + 1],
                                     scale=-1.0,
                                     accum_out=csum[:, ct:ct + 1])
            crcp = misc.tile([P, 4], f32)
            nc.vector.reciprocal(crcp[:], csum[:])
            nc.vector.tensor_scalar(out=crcp[:], in0=crcp[:],
                                    scalar1=gca[:, 0:1], scalar2=None,
                                    op0=OP.mult)
            cattn_bf = xbf_pool.tile([P, 4, NB], bf16, tag="xbf",
                                     name="cattn_bf")
            for ct in range(4):
                nc.vector.tensor_scalar(out=cattn_bf[:, ct, :],
                                        in0=cattn_sb[:, ct, :],
                                        scalar1=crcp[:, ct:ct + 1],
                                        scalar2=None, op0=OP.mult)

            # ---------------- D3: cattn^T (bf16) ----------------
            catT = catT_pool.tile([P, 4, NB], bf16)
            for dt in range(4):
                tp = pmm.tile([P, NB], bf16, tag="mm", name=f"catp{dt}")
                for ct in range(4):
                    nc.tensor.transpose(tp[:, ct * P:(ct + 1) * P],
                                        cattn_bf[:, ct, dt * P:(dt + 1) * P],
                                        ident_bf[:])
                nc.vector.tensor_copy(catT[:, dt, :], tp[:])

            # ---------------- E: position attention ----------------
            ysum = misc.tile([P, 4, 4], f32)
            ysq = misc.tile([P, 4, 4], f32)

            def emit_ca(g, ca2x):
                for p in range(16):
                    emit_ca_step(g, ca2x, p)
                return ca2x

            ca_state = {}

            def emit_ca_step(g, ca2x, p):
                gsc = slice(g * NB, (g + 1) * NB)
                ct, dt = divmod(p, 4)
                if dt == 0:
                    ca_state["cp"] = pmm.tile([P, NB], f32, tag="mm",
                                              name=f"cap{g}_{ct}")
                cp = ca_state["cp"]
                nc.tensor.matmul(cp[:], catT[:, dt, ct * P:(ct + 1) * P],
                                 xbf_all[:, dt, gsc],
                                 start=(dt == 0), stop=(dt == 3))
                if dt == 3:
                    nc.vector.tensor_tensor(out=ca2x[:, ct, :], in0=cp[:],
                                            in1=xbf_all[:, ct, gsc],
                                            op=OP.add)
                    nc.gpsimd.tensor_tensor(out=ca2x[:, ct, :],
                                            in0=ca2x[:, ct, :],
                                            in1=xbf_all[:, ct, gsc],
                                            op=OP.add)

            def emit_sq(g, ot):
                # deferred BN sum-of-squares, reading the SBUF y copy so it
                # never holds PSUM banks; ACT for even ot, DVE for odd
                gsf = slice(g * NB, (g + 1) * NB)
                if ot % 2 == 0:
                    sqb = small2.tile([P, NB], f32, tag="sq",
                                      name=f"sq{g}_{ot}")
                    nc.scalar.activation(out=sqb[:], in_=y_sb[:, ot, gsf],
                                         func=AF.Square, bias=0.0, scale=1.0,
                                         accum_out=ysq[:, ot, g:g + 1])
                else:
                    sqb = small2.tile([P, NB], f32, tag="sq",
                                      name=f"sq{g}_{ot}")
                    nc.vector.tensor_tensor(out=sqb[:], in0=y_sb[:, ot, gsf],
                                            in1=y_sb[:, ot, gsf], op=OP.mult)
                    nc.vector.tensor_reduce(out=ysq[:, ot, g:g + 1],
                                            in_=sqb[:], axis=AX, op=OP.add)

            ca_cur = w8k.tile([P, 4, NB], f32, tag="w8", name="ca0")
            emit_ca(0, ca_cur)
            sts = g0_sts
            defer_sq = []
            for g in range(NG):
                pv = [pacc.tile([P, NB], f32, tag="acc", name=f"pv{g}_{ct}")
                      for ct in range(4)]
                pr_ = prow.tile([P, NB], f32, tag="rs", name=f"rs{g}")
                ca_next = None
                if g + 1 < NG:
                    ca_next = w8k.tile([P, 4, NB], f32, tag="w8",
                                       name=f"ca{g + 1}")
                for p in range(PAIRS):
                    while len(sts) < min(p + 5, PAIRS):
                        sts.append(emit_pair(g, len(sts)))
                    st = sts[p]
                    for ct in range(4):
                        nc.tensor.matmul(
                            pv[ct],
                            vT8[:, 2 * p:2 * p + 2, ct * P:(ct + 1) * P],
                            st[:], start=(p == 0), stop=(p == PAIRS - 1),
                            perf_mode=DR)
                    nc.tensor.matmul(pr_, ones8[:], st[:],
                                     start=(p == 0), stop=(p == PAIRS - 1),
                                     perf_mode=DR)
                    # next group's ca chain + the previous group's deferred
                    # squares ride the PE/ACT/DVE slack of the pair loop
                    if ca_next is not None:
                        emit_ca_step(g + 1, ca_next, p)
                    if defer_sq and p in (4, 6, 8, 10):
                        defer_sq.pop(0)()

                # epilogue: ca(g+1) tail + next group's first pairs keep PE
                # busy under the DVE reciprocal; the s0 reads then release
                # the PV banks one-by-one, and this group's fuse matmuls
                # recycle them per-ot with an immediate alternating DVE/ACT
                # evac, so no separate fuse/evac stage blocks group g+1
                next_sts = []
                if g + 1 < NG:
                    next_sts = [emit_pair(g + 1, p) for p in range(4)]

                rr = small2.tile([P, NB], f32, tag="sq", name=f"rr{g}")
                nc.vector.reciprocal(rr[:], pr_)
                ca_g = ca_cur
                ca_cur = ca_next
                s_f = sfp.tile([P, 4, NB], f32r, tag="sf", name=f"sf{g}")
                s0 = w8k.tile([P, 4, NB], f32, tag="w8", name=f"s0_{g}")
                for ct in range(4):
                    nc.vector.tensor_tensor(out=s0[:, ct, :],
                                            in0=pv[ct], in1=rr[:],
                                            op=OP.mult)
                    nc.vector.tensor_tensor(out=s_f[:, ct, :],
                                            in0=s0[:, ct, :],
                                            in1=ca_g[:, ct, :], op=OP.add)

                gsf = slice(g * NB, (g + 1) * NB)
                for ot in range(4):
                    yp = pacc.tile([P, NB], f32, tag="acc",
                                   name=f"y{g}_{ot}")
                    for ct in range(4):
                        nc.tensor.matmul(yp,
                                         wf_r[:, ct, ot * P:(ot + 1) * P],
                                         s_f[:, ct, :],
                                         start=(ct == 0), stop=(ct == 3))
                    if ot % 2 == 0:
                        nc.vector.tensor_copy(y_sb[:, ot, gsf], yp)
                        nc.vector.tensor_reduce(out=ysum[:, ot, g:g + 1],
                                                in_=y_sb[:, ot, gsf],
                                                axis=AX, op=OP.add)
                    else:
                        nc.scalar.activation(out=y_sb[:, ot, gsf],
                                             in_=yp, func=AF.Copy,
                                             bias=0.0, scale=1.0,
                                             accum_out=ysum[:, ot, g:g + 1])
                    defer_sq.append(
                        (lambda gg, oo: lambda: emit_sq(gg, oo))(g, ot))
                sts = next_sts

            # last group's squares never got a pair loop to ride
            while defer_sq:
                defer_sq.pop(0)()

            # ---------------- F: BN via AllReduce + apply ----------------
            stats = misc.tile([P, 8, 1], f32)
            nc.vector.tensor_reduce(out=stats[:, 0:4, :],
                                    in_=ysum[:], axis=AX, op=OP.add)
            nc.vector.tensor_reduce(out=stats[:, 4:8, :],
                                    in_=ysq[:], axis=AX, op=OP.add)
            stats_f = stats.rearrange("p a b -> p (a b)")
            sin = dram.tile([P, 8], f32)
            sout = dram.tile([NCORES, P * 8], f32)
            nc.sync.dma_start(sin[:], stats_f)
            nc.gpsimd.collective_compute(
                "AllGather", OP.bypass,
                replica_groups=[list(range(NCORES))],
                ins=[sin.opt()], outs=[sout.opt()],
            )
            # per-rank contiguous [P,8] loads (the one-shot [P,8,R] gather
            # is 4B-strided and costs ~12us in DMA packets), then a short
            # DVE rank-sum tree
            gst = misc.tile([P, NCORES, 8], f32)
            sout_r = sout.rearrange("r (p j) -> r p j", p=P)
            for r in range(NCORES):
                q = nc.sync if r % 2 == 0 else nc.scalar
                q.dma_start(gst[:, r, :], sout_r[r])
            for r in range(1, NCORES):
                nc.vector.tensor_tensor(out=gst[:, 0, :], in0=gst[:, 0, :],
                                        in1=gst[:, r, :], op=OP.add)
            mean = misc.tile([P, 4], f32)
            msq = misc.tile([P, 4], f32)
            nc.vector.tensor_scalar(out=mean[:], in0=gst[:, 0, 0:4],
                                    scalar1=1.0 / CNT, scalar2=None,
                                    op0=OP.mult)
            nc.vector.tensor_scalar(out=msq[:], in0=gst[:, 0, 4:8],
                                    scalar1=1.0 / CNT, scalar2=None,
                                    op0=OP.mult)
            var = misc.tile([P, 4], f32)
            nc.vector.tensor_tensor(out=var[:], in0=mean[:], in1=mean[:],
                                    op=OP.mult)
            nc.vector.tensor_tensor(out=var[:], in0=msq[:], in1=var[:],
                                    op=OP.subtract)
            eps_t = misc.tile([P, 1], f32)
            nc.vector.memset(eps_t[:], BN_EPS)
            sd = misc.tile([P, 4], f32)
            nc.scalar.activation(out=sd[:], in_=var[:], func=AF.Sqrt,
                                 bias=eps_t[:, 0:1], scale=1.0)
            rstd = misc.tile([P, 4], f32)
            nc.vector.reciprocal(rstd[:], sd[:])
            a_t = misc.tile([P, 4], f32)
            nc.vector.tensor_tensor(out=a_t[:], in0=bnw[:], in1=rstd[:],
                                    op=OP.mult)
            b_t = misc.tile([P, 4], f32)
            nc.vector.tensor_tensor(out=b_t[:], in0=mean[:], in1=a_t[:],
                                    op=OP.mult)
            nc.vector.tensor_tensor(out=b_t[:], in0=bnb[:], in1=b_t[:],
                                    op=OP.subtract)

            # apply relu(y*a+b) alternating ACT/DVE; stores balanced 10:6
            # over the SP and ACT DMA queues
            for idx in range(16):
                ot, gg = divmod(idx, 4)
                ggs = slice(gg * NB, (gg + 1) * NB)
                ost = outst.tile([P, NB], f32, tag="ost",
                                 name=f"ost{ot}_{gg}")
                if idx % 2 == 0:
                    nc.scalar.activation(out=ost[:], in_=y_sb[:, ot, ggs],
                                         func=AF.Relu,
                                         scale=a_t[:, ot:ot + 1],
                                         bias=b_t[:, ot:ot + 1])
                else:
                    nc.vector.tensor_scalar(out=ost[:],
                                            in0=y_sb[:, ot, ggs],
                                            scalar1=a_t[:, ot:ot + 1],
                                            scalar2=b_t[:, ot:ot + 1],
                                            op0=OP.mult, op1=OP.add)
                    nc.vector.tensor_scalar(out=ost[:], in0=ost[:],
                                            scalar1=0.0, scalar2=None,
                                            op0=OP.max)
                store = nc.sync if idx % 8 < 5 else nc.scalar
                store.dma_start(y_ap[ot * P:(ot + 1) * P, ggs], ost[:])

    nc.compile()
    return nc


def _make_in_maps(inputs):
    import ml_dtypes
    x = np.asarray(inputs["x"], dtype=np.float32)
    xf = x.reshape(B, C, N)
    gpa_s = np.float32(np.asarray(inputs["gamma_pa"]).reshape(-1)[0])
    gca_s = np.float32(np.asarray(inputs["gamma_ca"]).reshape(-1)[0])
    wqt = np.ascontiguousarray(
        np.asarray(inputs["wq"], np.float32).T).astype(ml_dtypes.float8_e4m3)
    wkt = np.ascontiguousarray(
        np.asarray(inputs["wk"], np.float32).T).astype(ml_dtypes.float8_e4m3)
    wvt = np.ascontiguousarray(
        4.0 * gpa_s * np.asarray(inputs["wv"], np.float32).T
    ).astype(ml_dtypes.float8_e4m3)
    wft = np.ascontiguousarray(np.asarray(inputs["w_fuse"], np.float32).T)
    identb = np.eye(P, dtype=ml_dtypes.bfloat16)
    gca = np.full((P, 1), gca_s, np.float32)
    bnw = np.ascontiguousarray(
        np.asarray(inputs["bn_scale"], np.float32).reshape(4, P).T)
    bnb = np.ascontiguousarray(
        np.asarray(inputs["bn_bias"], np.float32).reshape(4, P).T)

    in_maps = []
    for c in range(NCORES):
        b, h = divmod(c, 2)
        own = xf[b][:, h * NH:(h + 1) * NH]
        other = xf[b][:, (1 - h) * NH:(2 - h) * NH]
        xl = np.ascontiguousarray(np.concatenate([own, other], axis=1))
        in_maps.append({
            "xl": xl, "wqt": wqt, "wkt": wkt, "wvt": wvt, "wft": wft,
            "identb": identb, "gca": gca, "bnw": bnw, "bnb": bnb,
        })
    return in_maps


def kernel(x, wq, wk, wv, w_fuse, gamma_pa, gamma_ca, bn_scale, bn_bias):
    from concourse.bass_utils import run_bass_kernel_spmd

    if "nc" not in _CACHE:
        _CACHE["nc"] = _build_program()
    nc = _CACHE["nc"]

    in_maps = _make_in_maps({
        "x": x, "wq": wq, "wk": wk, "wv": wv, "w_fuse": w_fuse,
        "gamma_pa": gamma_pa, "gamma_ca": gamma_ca,
        "bn_scale": bn_scale, "bn_bias": bn_bias,
    })
    res = run_bass_kernel_spmd(nc, in_maps, core_ids=list(range(NCORES)))
    out = np.empty((B, C, N), dtype=np.float32)
    for c in range(NCORES):
        b, h = divmod(c, 2)
        out[b][:, h * NH:(h + 1) * NH] = res.results[c]["y"]
    return out.reshape(B, C, H, W)
